# revision 33
# baseline (speedup 1.0000x reference)
"""Trainium2 Bass kernel for nn_Decoder (pre-LN transformer decoder layer).

Sharding: 8 cores = 4 batches x 2 sequence-halves. Core pid -> (batch=pid//2,
s=pid%2). s=0 handles query tokens [0,T0), s=1 handles [T0,L). Each core
computes k/v for its key range on its own (s=1 recomputes the prefix
projections), so no collectives are needed; the host concatenates outputs.

Layout strategy: activations token-major for LN/residual (per-partition
scalars via bn_stats), feature-major (transposed via bf16 PE transpose) for
the matmul chain. Attention computed fully on-chip flash-style:
scoresT [keys, q] -> exp (ACT, scale=1/HD) -> PV matmul with 64 ones-columns
(shared tail block addressed via a strided AP) so the softmax denominator
lands in PSUM partitions 64:127. All matmuls bf16 (weights cast during DMA),
everything else fp32.
"""
import os
import sys

sys.path.insert(0, "/opt/trn_rl_repo")

import contextlib

import numpy as np

import concourse.bass as bass
import concourse.mybir as mybir
import concourse.tile as tile
import concourse.tile_utils as tile_utils
from concourse import bacc
from concourse.bass_utils import run_bass_kernel_spmd
from concourse.masks import make_identity

# trn2 has 224KB/partition physical, ~208 usable; default cap is stale 192.
tile_utils.max_sbuf_usage = 206 * 1024

F32 = mybir.dt.float32
BF16 = mybir.dt.bfloat16
AF = mybir.ActivationFunctionType
ALU = mybir.AluOpType

if os.environ.get("DECODER_DIMS"):
    B, L, D, H, I, T0 = (int(v) for v in os.environ["DECODER_DIMS"].split(","))
else:
    B, L, D, H, I, T0 = 4, 2048, 768, 12, 3072, 1280
HD = 64
T1 = L - T0
EPS = 1e-5
N_CORES = 2 * B
ND = D // 128
NI = I // 128
NH = H
MASK_BIG = -1e9
BN_SUB = 256


def attn_spans(q_start, q_len, span=512):
    out = []
    q0 = q_start
    while q0 < q_start + q_len:
        w = min(span, q_start + q_len - q0)
        out.append((q0, w, q0 // 128))
        q0 += w
    return out


def build_body(nc, tc, ctx, io, q_start, q_len, kv_len):
    x, w_qkv, w_o, w1, w2, bqkv, bo, b1, b2, out = io
    NT_KV = kv_len // 128
    NT_Q = q_len // 128

    # ---------------- constant tiles ----------------
    consts = ctx.enter_context(tc.tile_pool(name="consts", bufs=1))
    ident = consts.tile([128, 128], BF16, tag="ident")
    make_identity(nc, ident[:])
    # tri[i, j] = 1 where query col j >= key row i (causal keep), else 0.
    # Applied multiplicatively to exp'd scores (post-exp mask).
    tri = consts.tile([128, 128], BF16, tag="tri")
    nc.vector.memset(tri[:], 1.0)
    nc.gpsimd.affine_select(
        out=tri[:], in_=tri[:], pattern=[[1, 128]],
        channel_multiplier=-1, base=0, compare_op=ALU.is_ge, fill=0.0)
    eps_t = consts.tile([128, 1], F32, tag="eps")
    nc.vector.memset(eps_t[:], EPS)

    def bcast(vec_ap, n, name, dtype=F32):
        t = consts.tile([128, n], dtype, tag=name)
        src = bass.AP(tensor=vec_ap.tensor, offset=vec_ap.offset,
                      ap=[[0, 128]] + vec_ap.ap)
        nc.gpsimd.dma_start(out=t[:], in_=src)
        return t

    bo_bc = bcast(bo, D, "bo_bc")          # f32: residual path
    b2_bc = bcast(b2, D, "b2_bc")          # f32: pre-gelu
    bv_src = bass.AP(tensor=bqkv.tensor, offset=bqkv.offset + 2 * HD,
                     ap=[[0, 128], [3 * HD, NH], [1, HD]])
    bv_bc = consts.tile([128, NH * HD], F32, tag="bv_bc")
    nc.gpsimd.dma_start(out=bv_bc[:], in_=bv_src)

    # Long-lived pools; left-side stack (open order) must be reverse of
    # close order: closes: ht + wearly (after A+B), qkv (after C), rest at
    # end. wffn lives on the RIGHT side so its lifetime (weight prefetch
    # during C through end of E) is independent of the left stack.
    oa_pool = ctx.enter_context(tc.tile_pool(name="oa_pool", bufs=1))
    h2_pool = ctx.enter_context(tc.tile_pool(name="h2_pool", bufs=2))
    at_cm = tc.tile_pool(name="attn_pool", bufs=1)
    at_pool = at_cm.__enter__()
    qkv_cm = tc.tile_pool(name="qkv_pool", bufs=1)
    qkv_pool = qkv_cm.__enter__()
    wearly_cm = tc.tile_pool(name="wearly", bufs=1)
    wearly = wearly_cm.__enter__()

    # ---------------- early weights (qkv) ----------------
    wqk = wearly.tile([128, ND, NH * 128], BF16, tag="wqk")
    for d in range(ND):
        src = bass.AP(tensor=w_qkv.tensor,
                      offset=w_qkv.offset + d * 128 * 3 * D,
                      ap=[[3 * D, 128], [3 * HD, NH], [1, 2 * HD]])
        nc.gpsimd.dma_start(out=wqk[:, d, :], in_=src)
    wv = wearly.tile([128, ND, NH * HD], BF16, tag="wv")
    for d in range(ND):
        src = bass.AP(tensor=w_qkv.tensor,
                      offset=w_qkv.offset + d * 128 * 3 * D + 2 * HD,
                      ap=[[3 * D, 128], [3 * HD, NH], [1, HD]])
        nc.gpsimd.dma_start(out=wv[:, d, :], in_=src)
    bqk = wearly.tile([128, NH], F32, tag="bqk")
    nc.gpsimd.dma_start(
        out=bqk[:],
        in_=bass.AP(tensor=bqkv.tensor, offset=bqkv.offset,
                    ap=[[1, 128], [3 * HD, NH]]))
    ht_cm = tc.tile_pool(name="ht_pool", bufs=1)
    ht_pool = ht_cm.__enter__()

    # -------- phases A+B fused per 512-token span: LN1 -> hT -> qkv --------
    # ln1_g/ln1_b (and ln2_g/ln2_b) are folded into w_qkv/b_qkv (w1/b1) on
    # the host, so LN here is just (x - mu) * rstd.
    hT = ht_pool.tile([128, ND, kv_len], BF16, tag="hT")
    # head h at partition half 64*(h%2), pair h//2, in both qT2 and kT2.
    qT2 = qkv_pool.tile([128, NH // 2, q_len], BF16, tag="qT2")
    kT2 = qkv_pool.tile([128, NH // 2, kv_len], BF16, tag="kT2")
    # v token-major: per (tok-window, head) a [128, 128] block of
    # [v (64 cols) | ones (64 cols)] so the PV matmul also produces the
    # softmax denominator on PSUM partitions 64:127.
    vaug = qkv_pool.tile([128, NT_KV, NH, 128], BF16, tag="vaug")

    def layernorm_tokmajor(x_t, pool, tagp):
        stats = pool.tile([128, D // BN_SUB, 6], F32, tag=tagp + "_stats")
        xs = x_t.rearrange("p (s c) -> p s c", c=BN_SUB)
        for sgi in range(D // BN_SUB):
            nc.vector.bn_stats(out=stats[:, sgi, :], in_=xs[:, sgi, :])
        mv = pool.tile([128, 2], F32, tag=tagp + "_mv")
        nc.vector.bn_aggr(out=mv[:], in_=stats[:])
        rstd = pool.tile([128, 1], F32, tag=tagp + "_rstd")
        nc.scalar.activation(out=rstd[:], in_=mv[:, 1:2],
                             func=AF.Sqrt, bias=eps_t[:], scale=1.0)
        nc.vector.reciprocal(out=rstd[:], in_=rstd[:])
        hb = pool.tile([128, D], BF16, tag=tagp + "_hb")
        nc.vector.tensor_scalar(
            out=hb[:], in0=x_t, scalar1=mv[:, 0:1], scalar2=rstd[:],
            op0=ALU.subtract, op1=ALU.mult)
        return hb

    with contextlib.ExitStack() as phAB:
        xpool = phAB.enter_context(tc.tile_pool(name="xpool", bufs=3))
        lnp = phAB.enter_context(tc.tile_pool(name="lnp", bufs=2))
        tpp = phAB.enter_context(
            tc.tile_pool(name="tpp", bufs=3, space="PSUM"))
        qkps = phAB.enter_context(
            tc.tile_pool(name="qkps", bufs=2, space="PSUM"))
        vps = phAB.enter_context(tc.tile_pool(name="vps", bufs=2, space="PSUM"))

        def emit_ln(sp0, w):
            for tw in range(sp0 // 128, (sp0 + w) // 128):
                x_t = xpool.tile([128, D], F32, tag="x_t")
                nc.sync.dma_start(out=x_t[:],
                                  in_=x[tw * 128:(tw + 1) * 128, :])
                hb = layernorm_tokmajor(x_t[:], lnp, "ln1")
                for d in range(ND):
                    pt = tpp.tile([128, 128], BF16, tag="tp_ps")
                    nc.tensor.transpose(pt[:], hb[:, d * 128:(d + 1) * 128],
                                        ident[:])
                    nc.vector.tensor_copy(
                        out=hT[:, d, tw * 128:(tw + 1) * 128], in_=pt[:])

        def emit_qk(sp0, w):
            for h in range(NH):
                hb2, hp = 64 * (h % 2), h // 2
                pq = qkps.tile([128, 512], F32, tag="pqk")
                for d in range(ND):
                    nc.tensor.matmul(
                        pq[:, 0:w], wqk[:, d, h * 128:(h + 1) * 128],
                        hT[:, d, sp0:sp0 + w],
                        start=(d == 0), stop=(d == ND - 1))
                # k-drain on ACT (Identity lives in every table: no reload)
                nc.scalar.activation(
                    out=kT2[hb2:hb2 + 64, hp, sp0:sp0 + w],
                    in_=pq[64:128, 0:w],
                    func=AF.Identity, bias=bqk[64:128, h:h + 1], scale=1.0)
                lo = max(sp0, q_start)
                hi = min(sp0 + w, q_start + q_len)
                if lo < hi:
                    nc.vector.tensor_scalar_add(
                        out=qT2[hb2:hb2 + 64, hp, lo - q_start:hi - q_start],
                        in0=pq[0:64, lo - sp0:hi - sp0],
                        scalar1=bqk[0:64, h:h + 1])

        def emit_v(sp0, w):
            half = NH * HD // 2
            nhh = NH // 2
            for tw in range(sp0 // 128, (sp0 + w) // 128):
                for hf in range(2):
                    pv = vps.tile([128, half], F32, tag="pv")
                    for d in range(ND):
                        nc.tensor.matmul(
                            pv[:], hT[:, d, tw * 128:(tw + 1) * 128],
                            wv[:, d, hf * half:(hf + 1) * half],
                            start=(d == 0), stop=(d == ND - 1))
                    dst = vaug[:, tw, hf * nhh:(hf + 1) * nhh, 0:HD]
                    bvs = bv_bc[:, hf * half:(hf + 1) * half].rearrange(
                        "p (h c) -> p h c", c=HD)
                    nc.vector.tensor_tensor(
                        out=dst, in0=pv[:].rearrange("p (h c) -> p h c", c=HD),
                        in1=bvs, op=ALU.add)
                nc.gpsimd.memset(vaug[:, tw, :, HD:128], 1.0)

        spans = [(s, min(512, kv_len - s)) for s in range(0, kv_len, 512)]
        emit_ln(*spans[0])
        for i, s in enumerate(spans):
            emit_qk(*s)
            if i + 1 < len(spans):
                emit_ln(*spans[i + 1])
            emit_v(*s)

    ht_cm.__exit__(None, None, None)
    wearly_cm.__exit__(None, None, None)

    # w_o/w1 weight prefetch: right-side pool opened only now (a right-side
    # pool reserves its whole arena at open) so the ~12MB of f32 weight
    # reads stream on the DMA engines underneath phase C; consumed in D/E.
    wffn_a_cm = tc.tile_pool(name="wffn_a", bufs=1, side="right")
    wffn = wffn_a_cm.__enter__()
    wo_sb = wffn.tile([128, ND, D], BF16, tag="wo_sb")
    for a in range(ND):
        nc.gpsimd.dma_start(out=wo_sb[:, a, :],
                            in_=w_o[a * 128:(a + 1) * 128, :])
    w1_sb = wffn.tile([128, ND, I], BF16, tag="w1_sb")
    for d in range(ND):
        nc.gpsimd.dma_start(out=w1_sb[:, d, :],
                            in_=w1[d * 128:(d + 1) * 128, :])
    b1_sb = wffn.tile([128, NI], F32, tag="b1_sb")
    nc.gpsimd.dma_start(
        out=b1_sb[:],
        in_=bass.AP(tensor=b1.tensor, offset=b1.offset,
                    ap=[[1, 128], [128, NI]]))

    # ------- phase C: attention + w_o/residual per span (span-outer) -------
    # Span-outer so each span ends with real w_o matmul work that fills the
    # PE while the next span's exp stream warms up -- keeps the HAM clock
    # gate at K=8 (2.4 GHz) instead of sagging to 1.2 GHz on a sparse
    # attention-only PE stream.
    attnT = at_pool.tile([128, ND, q_len], BF16, tag="attnT")
    oaT = oa_pool.tile([128, NT_Q, D], BF16, tag="oaT")
    Dh = D // 2

    with contextlib.ExitStack() as phC:
        sps = phC.enter_context(tc.tile_pool(name="sps", bufs=2, space="PSUM"))
        ops_ = phC.enter_context(
            tc.tile_pool(name="ops", bufs=2, space="PSUM"))
        wops = phC.enter_context(
            tc.tile_pool(name="wops", bufs=2, space="PSUM"))
        epool = phC.enter_context(tc.tile_pool(name="epool", bufs=3))
        rpool = phC.enter_context(tc.tile_pool(name="rpool", bufs=2))
        xpool2 = phC.enter_context(tc.tile_pool(name="xpool2", bufs=2))
        EW = 2  # key-tiles per exp group (2x2 banks + 2 po + 2 pw = 8)

        for (q0, w, nfull) in attn_spans(q_start, q_len):
            ndiag = w // 128
            ktot = nfull + ndiag
            for h in range(NH):
                hb2, hp = 64 * (h % 2), h // 2
                po = ops_.tile([128, 512], F32, tag="po")

                # Key-tiles (full and diagonal alike) run in groups of EW
                # sharing one PSUM tile and one ACT exp call. PV matmuls are
                # emitted one group late so the PE streams scores(g+1) while
                # ACT runs exp(g) -- no PE wait on the exp. Scalar engine
                # stays on the Exp table all phase; causal masking is a
                # cheap post-exp bf16 triangular multiply.
                def emit_pv(group):
                    et, tiles = group
                    for (k, jj, c0, wj) in tiles:
                        if k >= nfull:
                            nc.vector.tensor_tensor(
                                out=et[:, jj, 0:128], in0=et[:, jj, 0:128],
                                in1=tri[:], op=ALU.mult)
                        nc.tensor.matmul(
                            po[:, c0:w], vaug[:, k, h, :], et[:, jj, 0:wj],
                            start=(k == 0), stop=(k == ktot - 1))

                prev = None
                kt = 0
                while kt < ktot:
                    ng = min(EW, ktot - kt)
                    ps = sps.tile([128, EW, 512], F32, tag="ps")
                    tiles = []
                    for jj in range(ng):
                        k = kt + jj
                        c0 = max(0, 128 * (k - nfull))
                        wj = w - c0
                        tiles.append((k, jj, c0, wj))
                        nc.tensor.matmul(
                            ps[:, jj, 0:wj],
                            kT2[hb2:hb2 + 64, hp, k * 128:(k + 1) * 128],
                            qT2[hb2:hb2 + 64, hp,
                                q0 + c0 - q_start:q0 + w - q_start],
                            start=True, stop=True)
                    wmax = tiles[0][3]
                    et = epool.tile([128, EW, 512], BF16, tag="et")
                    nc.scalar.activation(out=et[:, 0:ng, 0:wmax],
                                         in_=ps[:, 0:ng, 0:wmax],
                                         func=AF.Exp, bias=0.0, scale=1.0 / HD)
                    if prev is not None:
                        emit_pv(prev)
                    prev = (et, tiles)
                    kt += ng
                emit_pv(prev)
                # 1/den on DVE (custom op, ~18-bit accurate, no ACT table)
                # den lives at PSUM partitions 64:128; shift-copy to 0:64
                # (single-input ops may shift partitions; two-input ops and
                # the custom-DVE recip may not), then fast reciprocal.
                dcp = rpool.tile([64, 512], F32, tag="dcp")
                nc.vector.tensor_copy(out=dcp[:, 0:w], in_=po[64:128, 0:w])
                rt = rpool.tile([64, 512], F32, tag="rt")
                nc.vector.reciprocal_approx_fast(out=rt[:, 0:w],
                                                 in_=dcp[:, 0:w])
                nc.vector.tensor_tensor(
                    out=attnT[64 * (h % 2):64 * (h % 2) + 64, hp,
                              q0 - q_start:q0 - q_start + w],
                    in0=po[0:64, 0:w], in1=rt[:, 0:w], op=ALU.mult)

            # D1 for this span: w_o + residual (+b_o) -> oaT (bf16)
            for twl in range(w // 128):
                tw = (q0 - q_start) // 128 + twl
                xo = xpool2.tile([128, D], F32, tag="xo")
                nc.sync.dma_start(
                    out=xo[:],
                    in_=x[q_start + tw * 128:q_start + (tw + 1) * 128, :])
                nc.vector.tensor_tensor(out=xo[:], in0=xo[:], in1=bo_bc[:],
                                        op=ALU.add)
                for hf in range(2):
                    pw = wops.tile([128, Dh], F32, tag="pw")
                    for a in range(ND):
                        nc.tensor.matmul(
                            pw[:], attnT[:, a, tw * 128:(tw + 1) * 128],
                            wo_sb[:, a, hf * Dh:(hf + 1) * Dh],
                            start=(a == 0), stop=(a == ND - 1))
                    nc.vector.tensor_tensor(
                        out=oaT[:, tw, hf * Dh:(hf + 1) * Dh], in0=pw[:],
                        in1=xo[:, hf * Dh:(hf + 1) * Dh], op=ALU.add)

    qkv_cm.__exit__(None, None, None)

    # w2 + ff1 pool deferred to here (SBUF too tight while qkv tiles live);
    # ff2 of chunk 0 only needs w2 ~40us into D/E, by which time the DMA
    # (~9.4MB f32 read) has streamed in.
    wffn_b_cm = tc.tile_pool(name="wffn_b", bufs=1, side="right")
    wffn_b = wffn_b_cm.__enter__()
    w2_sb = wffn_b.tile([128, NI, D], BF16, tag="w2_sb")
    for i_ in range(NI):
        nc.gpsimd.dma_start(out=w2_sb[:, i_, :],
                            in_=w2[i_ * 128:(i_ + 1) * 128, :])

    # ---------- phase E per 512-token chunk: LN2 -> h2T -> FFN ----------
    with contextlib.ExitStack() as phE:
        tpf = phE.enter_context(
            tc.tile_pool(name="tpf", bufs=2, space="PSUM"))
        # bufs=5: all 4 hb tiles of a chunk stay alive until the transposes
        lnp2 = phE.enter_context(tc.tile_pool(name="lnp2", bufs=5))
        f1ps = phE.enter_context(
            tc.tile_pool(name="f1ps", bufs=2, space="PSUM"))
        f2ps = phE.enter_context(
            tc.tile_pool(name="f2ps", bufs=2, space="PSUM"))
        opool = phE.enter_context(tc.tile_pool(name="opool", bufs=2))

        # LN2 stats for ALL tiles up front: one Sqrt table load total; the
        # per-chunk normalizes below are then DVE-only, so the scalar
        # engine holds the Gelu table for the whole FFN phase.
        mv2 = lnp2.tile([128, NT_Q, 2], F32, tag="ln2_mv2", bufs=1)
        rsds = lnp2.tile([128, NT_Q], F32, tag="ln2_rsds", bufs=1)
        for tw in range(NT_Q):
            stats = lnp2.tile([128, D // BN_SUB, 6], F32, tag="ln2_stats")
            xs = oaT[:, tw, :].rearrange("p (s c) -> p s c", c=BN_SUB)
            for sgi in range(D // BN_SUB):
                nc.vector.bn_stats(out=stats[:, sgi, :], in_=xs[:, sgi, :])
            nc.vector.bn_aggr(out=mv2[:, tw, :], in_=stats[:])
            nc.scalar.activation(out=rsds[:, tw:tw + 1], in_=mv2[:, tw, 1:2],
                                 func=AF.Sqrt, bias=eps_t[:], scale=1.0)
        nc.vector.reciprocal(out=rsds[:], in_=rsds[:])

        for c0 in range(0, q_len, 512):
            cw = min(512, q_len - c0)
            ctws = cw // 128
            h2T = h2_pool.tile([128, ND, 512], BF16, tag="h2T")
            hbs = []
            for twl in range(ctws):
                tw = c0 // 128 + twl
                hb = lnp2.tile([128, D], BF16, tag="ln2_hb")
                nc.vector.tensor_scalar(
                    out=hb[:], in0=oaT[:, tw, :], scalar1=mv2[:, tw, 0:1],
                    scalar2=rsds[:, tw:tw + 1],
                    op0=ALU.subtract, op1=ALU.mult)
                hbs.append(hb)
            # transposes: all 6 d-blocks of a tile share ONE 1-bank psum
            # tile (bf16), drained in a single strided copy
            for twl in range(ctws):
                ptf = tpf.tile([128, D], BF16, tag="ptf")
                for d in range(ND):
                    nc.tensor.transpose(
                        ptf[:, d * 128:(d + 1) * 128],
                        hbs[twl][:, d * 128:(d + 1) * 128], ident[:])
                nc.vector.tensor_copy(
                    out=h2T[:, :, twl * 128:(twl + 1) * 128],
                    in_=ptf[:].rearrange("p (d t) -> p d t", t=128))
            ff1 = wffn_b.tile([128, NI, 512], BF16, tag="ff1")
            for i_ in range(NI):
                pf = f1ps.tile([128, 512], F32, tag="pf1")
                for d in range(ND):
                    nc.tensor.matmul(
                        pf[:, 0:cw], w1_sb[:, d, i_ * 128:(i_ + 1) * 128],
                        h2T[:, d, 0:cw],
                        start=(d == 0), stop=(d == ND - 1))
                nc.vector.tensor_scalar_add(
                    out=ff1[:, i_, 0:cw], in0=pf[:, 0:cw],
                    scalar1=b1_sb[:, i_:i_ + 1])
            for twl in range(ctws):
                tw = c0 // 128 + twl
                ot = opool.tile([128, D], F32, tag="ot")
                for hf in range(2):
                    pg = f2ps.tile([128, Dh], F32, tag="pf2")
                    for i_ in range(NI):
                        nc.tensor.matmul(
                            pg[:], ff1[:, i_, twl * 128:(twl + 1) * 128],
                            w2_sb[:, i_, hf * Dh:(hf + 1) * Dh],
                            start=(i_ == 0), stop=(i_ == NI - 1))
                    sl = slice(hf * Dh, (hf + 1) * Dh)
                    gb = opool.tile([128, Dh], F32, tag="gb")
                    nc.vector.tensor_tensor(out=gb[:], in0=pg[:],
                                            in1=b2_bc[:, sl], op=ALU.add)
                    nc.scalar.activation(out=gb[:], in_=gb[:], func=AF.Gelu,
                                         bias=0.0, scale=1.0)
                    nc.vector.tensor_tensor(out=ot[:, sl], in0=gb[:],
                                            in1=oaT[:, tw, sl],
                                            op=ALU.add)
                nc.sync.dma_start(out=out[tw * 128:(tw + 1) * 128, :],
                                  in_=ot[:])

    wffn_b_cm.__exit__(None, None, None)
    wffn_a_cm.__exit__(None, None, None)
    at_cm.__exit__(None, None, None)


_NC_CACHE = {}


def build_kernel():
    key = (B, L, D, H, I, T0)
    if key in _NC_CACHE:
        return _NC_CACHE[key]
    nc = bacc.Bacc("TRN2", target_bir_lowering=False, debug=False,
                   num_devices=N_CORES)
    x = nc.dram_tensor("x", [L, D], F32, kind="ExternalInput").ap()
    w_qkv = nc.dram_tensor("w_qkv", [D, 3 * D], F32, kind="ExternalInput").ap()
    w_o = nc.dram_tensor("w_o", [D, D], F32, kind="ExternalInput").ap()
    w1 = nc.dram_tensor("w1", [D, I], F32, kind="ExternalInput").ap()
    w2 = nc.dram_tensor("w2", [I, D], F32, kind="ExternalInput").ap()
    bqkv = nc.dram_tensor("b_qkv", [3 * D], F32, kind="ExternalInput").ap()
    bo = nc.dram_tensor("b_o", [D], F32, kind="ExternalInput").ap()
    b1 = nc.dram_tensor("b1", [I], F32, kind="ExternalInput").ap()
    b2 = nc.dram_tensor("b2", [D], F32, kind="ExternalInput").ap()
    out = nc.dram_tensor("out", [T0, D], F32, kind="ExternalOutput").ap()
    io = (x, w_qkv, w_o, w1, w2, bqkv, bo, b1, b2, out)

    pid = nc.partition_id()
    with tile.TileContext(nc) as tc:
        with tc.If(pid % 2 == 0):
            with contextlib.ExitStack() as c0:
                build_body(nc, tc, c0, io, 0, T0, T0)
        with tc.If(pid % 2 == 1):
            with contextlib.ExitStack() as c1:
                build_body(nc, tc, c1, io, T0, T1, L)
    nc.compile()
    _NC_CACHE[key] = nc
    return nc


def make_in_maps(inputs):
    """Fold LN gains/biases into the adjacent projection weights (exact in
    fp32 terms) and build the per-core input maps."""
    x = np.asarray(inputs["x"], dtype=np.float32)
    am = np.asarray(inputs["attention_mask"])
    assert am.all(), "kernel assumes attention_mask all-True (spec fill=ones)"
    g = {n: np.asarray(inputs[n], np.float64)
         for n in ["w_qkv", "b_qkv", "w_o", "b_o", "w1", "b1", "w2", "b2",
                   "ln1_g", "ln1_b", "ln2_g", "ln2_b"]}
    common = {
        "w_qkv": g["ln1_g"][:, None] * g["w_qkv"],
        "b_qkv": g["ln1_b"] @ g["w_qkv"] + g["b_qkv"],
        "w_o": g["w_o"], "b_o": g["b_o"],
        "w1": g["ln2_g"][:, None] * g["w1"],
        "b1": g["ln2_b"] @ g["w1"] + g["b1"],
        "w2": g["w2"], "b2": g["b2"],
    }
    common = {k: np.ascontiguousarray(v, dtype=np.float32)
              for k, v in common.items()}
    in_maps = []
    for pid in range(N_CORES):
        b = pid // 2
        m = dict(common)
        m["x"] = np.ascontiguousarray(x[b])
        in_maps.append(m)
    return in_maps


def kernel(**inputs):
    nc = build_kernel()
    in_maps = make_in_maps(inputs)
    res = run_bass_kernel_spmd(nc, in_maps, core_ids=list(range(N_CORES)))
    out = np.empty((B, L, D), np.float32)
    for b in range(B):
        out[b, :T0] = res.results[2 * b]["out"][:T0]
        out[b, T0:] = res.results[2 * b + 1]["out"][:T1]
    return out


if __name__ == "__main__":
    rng = np.random.default_rng(0)
    ins = {
        "x": rng.standard_normal((B, L, D)).astype(np.float32),
        "attention_mask": np.ones((B, L), bool),
        "ln1_g": np.ones(D, np.float32), "ln1_b": np.zeros(D, np.float32),
        "w_qkv": (rng.standard_normal((D, 3 * D)) * 0.02).astype(np.float32),
        "b_qkv": np.zeros(3 * D, np.float32),
        "w_o": (rng.standard_normal((D, D)) * 0.02).astype(np.float32),
        "b_o": np.zeros(D, np.float32),
        "ln2_g": np.ones(D, np.float32), "ln2_b": np.zeros(D, np.float32),
        "w1": (rng.standard_normal((D, I)) * 0.02).astype(np.float32),
        "b1": np.zeros(I, np.float32),
        "w2": (rng.standard_normal((I, D)) * 0.02).astype(np.float32),
        "b2": np.zeros(D, np.float32),
    }
    o = kernel(**ins)
    print("kernel out:", o.shape, o.dtype, np.abs(o).max())

